# revision 24
# baseline (speedup 1.0000x reference)
"""Trainium2 Bass kernel for nn_BertGNNGru (attention-gated GRU scan).

Strategy (data-parallel over batch, 8 cores x 128 rows):
  - Fold the attention gate algebraically into the GRU weight matrices:
      inputgate = sigmoid([i_i|h_i] @ Wa.T + ba)
                = sigmoid(x@(Wa_i@Wx_i).T + h@(Wa_h@Wh_i).T + const)
    so the whole step becomes one 768-row "projection" from x and one from h.
  - Keep the recurrence in transposed form: hidden state lives as
    hyT [H-on-partitions, batch-on-free], which makes it directly usable as
    the moving operand of the next step's matmuls -> no per-step transposes.
  - All matmuls in bf16 (1 cycle/column on the PE); x is pre-cast to bf16 in
    a DRAM scratch buffer (SWDGE cast-DMA), and per-step xT tiles are produced
    by the DMA crossbar transpose engine (issued from the otherwise idle SP).
  - Per-feature biases ride for free as ACT per-partition bias operands.
  - Output is transposed back to [batch, H] by the PE (transpose matmul) and
    upcast bf16->fp32 during the DMA store (SWDGE cast).
"""

import os
from contextlib import ExitStack

import numpy as np
import ml_dtypes

import concourse.bass as bass
import concourse.tile as tile
from concourse import bacc, mybir
from concourse import bass_utils

F32 = mybir.dt.float32
BF16 = mybir.dt.bfloat16

B, T_FULL, D, H = 1024, 512, 256, 256
NCORES = 8
BS = B // NCORES          # 128 batch rows per core
G3 = 3 * H                # 768 folded projection rows: [r | a | n]
YGRP = 8                  # steps per output store group
XCH_STEPS = 32            # steps per x-cast chunk
ADD, SUB, MUL = mybir.AluOpType.add, mybir.AluOpType.subtract, mybir.AluOpType.mult
SIG, TANH = mybir.ActivationFunctionType.Sigmoid, mybir.ActivationFunctionType.Tanh


def _emit(ctx: ExitStack, tc: tile.TileContext, x_d, wpx_d, wph_d, bias_d, y_d, T):
    nc = tc.nc
    abl = os.environ.get("GRU_ABL", "none")
    # abl may be comma-separated; membership test via split
    abl_set = set(abl.split(","))
    from concourse.masks import make_identity

    # ---------------- pools ----------------
    wpool = ctx.enter_context(tc.tile_pool(name="w", bufs=1))
    xstage = ctx.enter_context(tc.tile_pool(name="xstage", bufs=2))
    dram = ctx.enter_context(tc.tile_pool(name="dram", bufs=1, space="DRAM"))
    xt_pool = ctx.enter_context(tc.tile_pool(name="xt", bufs=4))
    ew = ctx.enter_context(tc.tile_pool(name="ew", bufs=3))
    hy_pool = ctx.enter_context(tc.tile_pool(name="hy", bufs=4))
    yrow_pool = ctx.enter_context(tc.tile_pool(name="yrow", bufs=2))
    psA = ctx.enter_context(tc.tile_pool(name="psA", bufs=3, space="PSUM"))
    psB = ctx.enter_context(tc.tile_pool(name="psB", bufs=3, space="PSUM"))
    psY = ctx.enter_context(tc.tile_pool(name="psY", bufs=2, space="PSUM"))

    # ---------------- constants ----------------
    wpx_sb = []
    wph_sb = []
    for k in range(2):
        wx = wpool.tile([128, G3], BF16, tag=f"wpx{k}")
        nc.sync.dma_start(wx[:], wpx_d[k])
        wpx_sb.append(wx)
        wh = wpool.tile([128, G3], BF16, tag=f"wph{k}")
        nc.sync.dma_start(wh[:], wph_d[k])
        wph_sb.append(wh)
    bias_sb = wpool.tile([128, 8], F32, tag="bias")
    nc.sync.dma_start(bias_sb[:], bias_d[:, :])
    ident = wpool.tile([128, 128], BF16, tag="ident")
    make_identity(nc, ident[:])
    ident2 = wpool.tile([128, 256], BF16, tag="ident2")
    nc.gpsimd.memset(ident2[:], 0.0)

    def bias_col(j):
        return bias_sb[:, j : j + 1]

    # ---------------- x cast prologue: fp32 DRAM -> bf16 DRAM scratch -------
    x_flat = x_d.rearrange("b t d -> b (t d)")  # [128, T*D]
    xch_steps = min(XCH_STEPS, T)
    n_chunks = (T + xch_steps - 1) // xch_steps
    ch_elems = xch_steps * D
    xbf_chunks = []
    for c in range(n_chunks):
        xc_dram = dram.tile([128, ch_elems], BF16, tag=f"xbf{c}")
        stage = xstage.tile([128, ch_elems], BF16, tag="xstage")
        nc.gpsimd.dma_start(stage[:], x_flat[:, c * ch_elems : (c + 1) * ch_elems])
        nc.sync.dma_start(xc_dram[:], stage[:])
        xbf_chunks.append(xc_dram)

    # ---------------- per-step state ----------------
    LOOKAHEAD = 2
    pA = {}  # step -> PSUM [128, 512] = [pre_r0 | pre_r1 | pre_a0 | pre_a1]
    pB = {}  # step -> PSUM [128, 512] = [ghn0 | ghn1 | pxn0 | pxn1]
    xT = {}  # step -> (tile_k0, tile_k1)
    hyT = {}  # step -> (tile_j0, tile_j1)
    yrow = {}  # group -> SBUF [128, YGRP*H] bf16

    def x_phase(s):
        """xT loads via DMA-crossbar transpose + x-side matmuls for step s."""
        c, off = divmod(s, xch_steps)
        xts = []
        for k in range(2):
            if "nox" in abl_set:
                xts.append(ident)
                continue
            t = xt_pool.tile([128, 128], BF16, tag=f"xt{k}", name=f"xt{k}")
            src = xbf_chunks[c][:, off * D + k * 128 : off * D + (k + 1) * 128]
            nc.sync.dma_start_transpose(t[:], src)
            xts.append(t)
        xT[s] = xts
        a = psA.tile([128, 512], F32, tag="pA")
        b = psB.tile([128, 512], F32, tag="pB")
        pA[s] = a
        pB[s] = b
        # x-side matmuls. Each PSUM bank is ONE accumulation group: start only
        # on the very first matmul into the bank, stop only on the last. The
        # h-side matmuls (2 iterations later) close the groups, except at s==0
        # where h == 0 and the x-side closes them itself.
        for k in range(2):
            for g in range(4):  # r0 r1 a0 a1
                nc.tensor.matmul(
                    a[:, g * 128 : (g + 1) * 128],
                    wpx_sb[k][:, g * 128 : (g + 1) * 128],
                    xts[k][:],
                    start=(k == 0 and g == 0),
                    stop=(s == 0 and k == 1 and g == 3),
                )
            for gi, g in enumerate((4, 5)):  # pxn0 pxn1 -> pB cols 256:512
                nc.tensor.matmul(
                    b[:, 256 + gi * 128 : 256 + (gi + 1) * 128],
                    wpx_sb[k][:, g * 128 : (g + 1) * 128],
                    xts[k][:],
                    start=(k == 0 and gi == 0),
                    stop=(s == 0 and k == 1 and gi == 1),
                )

    def out_phase(th):
        """Transpose step th's hyT back to [batch, H] and stage for the store."""
        g, slot = divmod(th, YGRP)
        if slot == 0:
            yrow[g] = yrow_pool.tile([128, YGRP * H], BF16, tag="yrow", name="yrow")
        yp = psY.tile([128, 256], BF16, tag="psY", name="yp")
        for j in range(2):
            nc.tensor.transpose(
                yp[:, j * 128 : (j + 1) * 128],
                hyT[th][:, j * 128 : (j + 1) * 128],
                ident[:],
            )
        dst = yrow[g]
        base = slot * H
        # one merged PSUM->SBUF copy; engine choice via GRU_YCOPY
        yc = os.environ.get("GRU_YCOPY", "dve")
        if yc == "act" or (yc == "alt" and th % 2 == 0):
            nc.scalar.copy(dst[:, base : base + 256], yp[:])
        else:
            nc.vector.tensor_copy(dst[:, base : base + 256], yp[:])
        if slot == YGRP - 1:
            t0 = th - (YGRP - 1)
            ydst = y_d[:, t0 : th + 1, :].rearrange("b t h -> b (t h)")
            nc.gpsimd.dma_start(ydst, yrow[g][:])  # bf16 -> fp32 cast store

    def h_phase(th):
        a, b = pA[th], pB[th]
        if th > 0:
            hprev = hyT[th - 1]
            if "nochain" in abl_set:
                hprev = ident2
            # interleave so the j=0 chain (r0, ghn0) unblocks earliest; the
            # banks were started by the x-side, so start=False everywhere and
            # stop only on the final matmul into each bank.
            for g, dst, col, last in (
                (0, a, 0, False),  # pre_r0
                (4, b, 0, False),  # ghn0
                (1, a, 128, False),  # pre_r1
                (5, b, 128, True),  # ghn1 -> closes pB
                (2, a, 256, False),  # pre_a0
                (3, a, 384, True),  # pre_a1 -> closes pA
            ):
                for k in range(2):
                    nc.tensor.matmul(
                        dst[:, col : col + 128],
                        wph_sb[k][:, g * 128 : (g + 1) * 128],
                        hprev[:, k * 128 : (k + 1) * 128],
                        start=False,
                        stop=(last and k == 1),
                    )
        # ---- output path for the previous step (PE + copies, off-chain) ----
        if th > 0 and "noy" not in abl_set:
            out_phase(th - 1)

        # ---- elementwise ----
        # merged [128, 256] container tiles; per-half ops where a per-chunk
        # bias (ACT) or chain pipelining (DVE) wants it, merged ops elsewhere.
        r = ew.tile([128, 256], BF16, tag="r", name="r")
        z = ew.tile([128, 256], BF16, tag="z", name="z")
        zp = ew.tile([128, 256], BF16, tag="zp", name="zp")
        t3 = ew.tile([128, 256], BF16, tag="t3", name="t3")
        u = ew.tile([128, 256], BF16, tag="u", name="u")
        t1 = ew.tile([128, 256], BF16, tag="t1", name="t1")
        n = ew.tile([128, 256], BF16, tag="n", name="n")
        hy = hy_pool.tile([128, 256], BF16, tag="hy", name="hy")

        def half(t_, j):
            return t_[:, j * 128 : (j + 1) * 128]

        for j in range(2):
            nc.scalar.activation(half(r, j), a[:, j * 128 : (j + 1) * 128], SIG,
                                 bias=bias_col(0 + j))
        for j in range(2):
            nc.scalar.activation(half(z, j), a[:, 256 + j * 128 : 256 + (j + 1) * 128],
                                 SIG, bias=bias_col(2 + j))
        for j in range(2):
            # zp_j = 1 - z_j ; t3_j = z_j * h_prev_j  (POOL, per half so w_j
            # doesn't wait on the other half's z)
            nc.gpsimd.tensor_scalar(half(zp, j), half(z, j), -1.0, 1.0, MUL, ADD)
            if th > 0:
                nc.gpsimd.tensor_tensor(half(t3, j), half(z, j),
                                        hyT[th - 1][:, j * 128 : (j + 1) * 128], MUL)
        for j in range(2):
            if th > 0:
                # u = (ghn + bh_n) * r
                nc.vector.scalar_tensor_tensor(
                    half(u, j), b[:, j * 128 : (j + 1) * 128], bias_col(4 + j),
                    half(r, j), ADD, MUL)
            else:
                nc.vector.tensor_scalar(half(u, j), half(r, j), bias_col(4 + j),
                                        None, MUL)
        for j in range(2):
            nc.vector.tensor_tensor(half(t1, j), half(u, j),
                                    b[:, 256 + j * 128 : 256 + (j + 1) * 128], ADD)
        for j in range(2):
            nc.scalar.activation(half(n, j), half(t1, j), TANH, bias=bias_col(6 + j))
        for j in range(2):
            if th > 0:
                w = ew.tile([128, 128], BF16, tag=f"ww{j}", name=f"ww{j}")
                nc.vector.tensor_tensor(w[:], half(n, j), half(zp, j), MUL)
                nc.vector.tensor_tensor(half(hy, j), w[:], half(t3, j), ADD)
            else:
                nc.vector.tensor_tensor(half(hy, j), half(n, j), half(zp, j), MUL)
        hyT[th] = hy
        # bookkeeping only; tile lifetimes are managed by the pools
        hyT.pop(th - 2, None)
        pA.pop(th, None)
        pB.pop(th, None)
        xT.pop(th, None)

    # ---------------- main loop ----------------
    for s in range(T):
        x_phase(s)
        if s >= LOOKAHEAD:
            h_phase(s - LOOKAHEAD)
    for th in range(T - LOOKAHEAD, T):
        h_phase(th)
    if "noy" not in abl_set:
        out_phase(T - 1)


def _build(T):
    nc = bacc.Bacc(
        "TRN2",
        target_bir_lowering=False,
        debug=False,
        num_devices=NCORES,
    )
    x_d = nc.dram_tensor("x", [BS, T, D], F32, kind="ExternalInput").ap()
    wpx_d = nc.dram_tensor("wpx", [2, 128, G3], BF16, kind="ExternalInput").ap()
    wph_d = nc.dram_tensor("wph", [2, 128, G3], BF16, kind="ExternalInput").ap()
    bias_d = nc.dram_tensor("bias", [128, 8], F32, kind="ExternalInput").ap()
    y_d = nc.dram_tensor("y", [BS, T, H], F32, kind="ExternalOutput").ap()
    with tile.TileContext(nc) as tc:
        with ExitStack() as ctx:
            _emit(ctx, tc, x_d, wpx_d, wph_d, bias_d, y_d, T)
    nc.compile()
    return nc


def _host_prep(Wx, bx, Wh, bh, Wa, ba):
    """Fold the attention gate into 768-row projection matrices (fp32 math)."""
    Wx_r, Wx_i, Wx_n = Wx[:H], Wx[H : 2 * H], Wx[2 * H :]
    Wh_r, Wh_i, Wh_n = Wh[:H], Wh[H : 2 * H], Wh[2 * H :]
    Wa_i, Wa_h = Wa[:, :H], Wa[:, H:]
    Wxa = Wa_i @ Wx_i
    Wha = Wa_h @ Wh_i
    bias_r = bx[:H] + bh[:H]
    bias_a = ba + Wa_i @ bx[H : 2 * H] + Wa_h @ bh[H : 2 * H]
    bh_n = bh[2 * H :]
    bx_n = bx[2 * H :]
    Wpx = np.concatenate([Wx_r, Wxa, Wx_n], axis=0)  # [768, 256]
    Wph = np.concatenate([Wh_r, Wha, Wh_n], axis=0)  # [768, 256]
    wpx = np.ascontiguousarray(
        Wpx.T.reshape(2, 128, G3).astype(ml_dtypes.bfloat16)
    )
    wph = np.ascontiguousarray(
        Wph.T.reshape(2, 128, G3).astype(ml_dtypes.bfloat16)
    )
    bias = np.stack(
        [
            bias_r[:128], bias_r[128:],
            bias_a[:128], bias_a[128:],
            bh_n[:128], bh_n[128:],
            bx_n[:128], bx_n[128:],
        ],
        axis=1,
    ).astype(np.float32)  # [128, 8]
    return wpx, wph, bias


def kernel(x, Wx, bx, Wh, bh, Wa, ba):
    x = np.ascontiguousarray(np.asarray(x, dtype=np.float32))
    Wx, bx, Wh, bh, Wa, ba = (
        np.asarray(a, dtype=np.float32) for a in (Wx, bx, Wh, bh, Wa, ba)
    )
    T = x.shape[1]
    wpx, wph, bias = _host_prep(Wx, bx, Wh, bh, Wa, ba)
    nc = _build(T)
    in_maps = []
    for c in range(NCORES):
        in_maps.append(
            {
                "x": np.ascontiguousarray(x[c * BS : (c + 1) * BS]),
                "wpx": wpx,
                "wph": wph,
                "bias": bias,
            }
        )
    res = bass_utils.run_bass_kernel_spmd(
        nc,
        in_maps,
        core_ids=list(range(NCORES)),
        trace=bool(int(os.environ.get("GRU_TRACE", "0"))),
    )
    global LAST_RESULTS
    LAST_RESULTS = res
    out = np.concatenate([res.results[c]["y"] for c in range(NCORES)], axis=0)
    return out.astype(np.float32)


LAST_RESULTS = None


if __name__ == "__main__":
    # smoke test with random data at reduced T
    Tt = int(os.environ.get("GRU_T", "16"))
    rng = np.random.default_rng(0)
    std = 1.0 / np.sqrt(H)
    x = rng.standard_normal((B, Tt, D), dtype=np.float32)
    u = lambda shape: rng.uniform(-std, std, shape).astype(np.float32)
    args = dict(
        x=x, Wx=u((G3, D)), bx=u((G3,)), Wh=u((G3, H)), bh=u((G3,)),
        Wa=u((H, 2 * H)), ba=u((H,)),
    )
    out = kernel(**args)
    # numpy reference
    def ref(x, Wx, bx, Wh, bh, Wa, ba):
        h = np.zeros((B, H), np.float32)
        outs = np.empty((B, Tt, H), np.float32)
        for t in range(Tt):
            gx = x[:, t] @ Wx.T + bx
            gh = h @ Wh.T + bh
            r = 1 / (1 + np.exp(-(gx[:, :H] + gh[:, :H])))
            att = np.concatenate([gx[:, H : 2 * H], gh[:, H : 2 * H]], 1)
            z = 1 / (1 + np.exp(-(att @ Wa.T + ba)))
            n = np.tanh(gx[:, 2 * H :] + r * gh[:, 2 * H :])
            hy = n + z * (h - n)
            h = hy
            outs[:, t] = hy
        return outs

    expected = ref(**args)
    err = np.linalg.norm(out - expected) / np.linalg.norm(expected)
    print("rel_l2 =", err)
    print("maxabs =", np.abs(out - expected).max(), "ref absmax", np.abs(expected).max())
